# revision 1
# baseline (speedup 1.0000x reference)
"""Multi-head attention kernel for 8 TRN2 NeuronCores.

Problem: x(4,2048,1024) -> MHA(16 heads, d=64) -> out(4,2048,1024), f32.

Sharding: core c handles (batch b = c//2, seq half = c%2): it computes
attention outputs (incl. all projections) for its 1024 query rows over all 16
heads.  K/V projections for the full batch are computed locally per core (2x
redundant) which keeps cores fully independent - zero collectives.

Layouts: everything contracts over SBUF partitions.  Host pre-transposes x and
weights.  Scores are computed as ST[j,i] = K_h Q_h^T so the softmax exp runs
on ScalarE straight out of PSUM with the 1/8 scale fused; softmax denominators
come for free from a ones-column appended to each V tile (M=65 PV matmul, row
64 = rowsum).  Normalization is reciprocal + a DRAM-bounce partition-broadcast
multiplied during the PSUM->SBUF evacuation.  V-projection bias is folded into
the output bias on host (bo_eff = bo + wo@bv).  Matmuls run in float32r
(TF32-ish rounding, ~1.6e-4 rel err end to end, full PE rate at N>=256).

Schedule: Q projection -> V projection (wv prefetched into the att slot, wot
into the wq slot) -> per head pair: K projection (SBUF-resident, fills PE
while softmax keeps ScalarE busy) + attention -> output projection (first
row-half starts during the last pair).  Staging writes ride the gpsimd SWDGE
queues so they never queue behind HWDGE prefetch reads.  This walrus build
accepts only ONE sync-wait per instruction, so a post-pass splits multi-wait
instructions into single-wait NoOps (_split_multi_waits), and custom DVE ops /
gpsimd ucode are unavailable (hence the DMA-based broadcast).
"""

import numpy as np
from contextlib import ExitStack

P = 128
EMB = 1024
SEQ = 2048
QR = 1024          # query rows per core
NH = 16
HD = 64
EC = EMB // P      # 8 contraction chunks
RC = SEQ // P      # 16 seq row chunks
NB = 512           # free-dim block
SCALE = 0.125      # 1/sqrt(64)

_COMPILED = None   # (nc, names) cache


def _patch_tile_drain():
    """This walrus build only accepts ONE sync-wait per Drain instruction; the
    stock TileContext tail drain carries one wait per pending proc.  Split it
    into a chain of single-wait drains."""
    import concourse.tile as tile
    from concourse.vector_clock import ScopedClock, VectorClock

    if getattr(tile.TileContext, "_drain_patched", False):
        return

    def _drain_and_barrier(self, tick_clock, wait_clock):
        nc = self.nc
        gc = tick_clock.global_clock
        vals = eval(repr(gc).replace("VectorClock", ""))
        n = len(vals)
        for i, v in enumerate(vals):
            if v > 0:
                sub = VectorClock([vals[j] if j == i else 0 for j in range(n)])
                d = nc.sync.drain()
                wait_clock.add_sem_waits(d.ins, ScopedClock({None: sub}))
        nc.all_engine_barrier()
        popped = nc._tile_sem_poison_stack.pop()
        assert popped is self._sem_poison
        nc.clear_and_free_semaphores(list(self.sems.allocated().values()))
        nc.all_engine_barrier()

    tile.TileContext._drain_and_barrier = _drain_and_barrier
    tile.TileContext._drain_patched = True


def _build():
    import concourse.bass as bass
    import concourse.mybir as mybir
    import concourse.tile as tile

    _patch_tile_drain()

    f32 = mybir.dt.float32
    f32r = mybir.dt.float32r
    Exp = mybir.ActivationFunctionType.Exp

    nc = bass.Bass()

    # xt holds this core's batch transposed, with the core's 1024 query rows
    # FIRST (host pre-permutes; key/value row order is irrelevant to MHA).
    xt = nc.dram_tensor("xt", [EMB, SEQ], f32r, kind="ExternalInput")
    wqt = nc.dram_tensor("wqt", [EMB, EMB], f32r, kind="ExternalInput")
    wkt = nc.dram_tensor("wkt", [EMB, EMB], f32r, kind="ExternalInput")
    wvt = nc.dram_tensor("wvt", [EMB, EMB], f32r, kind="ExternalInput")
    wot = nc.dram_tensor("wot", [EMB, EMB], f32r, kind="ExternalInput")
    bqp = nc.dram_tensor("bqp", [P, EC], f32, kind="ExternalInput")
    bkp = nc.dram_tensor("bkp", [P, EC], f32, kind="ExternalInput")
    bob = nc.dram_tensor("bob", [P, EMB], f32, kind="ExternalInput")
    out = nc.dram_tensor("out", [QR, EMB], f32, kind="ExternalOutput")

    qstage = nc.dram_tensor("qstage", [EC, P, QR], f32r, kind="Internal")
    vstage = nc.dram_tensor("vstage", [RC, P, EMB], f32r, kind="Internal")
    # softmax denominator bounce buffer (for partition-broadcast via DMA)
    bscr = nc.dram_tensor("bscr", [NH, 2, NB], f32, kind="Internal")

    with tile.TileContext(nc) as tc, ExitStack() as ctx:
        big = ctx.enter_context(tc.tile_pool(name="big", bufs=1))
        wpool = ctx.enter_context(tc.tile_pool(name="w", bufs=1))
        pspool = ctx.enter_context(tc.tile_pool(name="ps", bufs=2, space="PSUM"))
        stpool = ctx.enter_context(tc.tile_pool(name="st", bufs=2, space="PSUM"))
        otpool = ctx.enter_context(tc.tile_pool(name="ot", bufs=2, space="PSUM"))
        evac = ctx.enter_context(tc.tile_pool(name="evac", bufs=3))
        ptpool = ctx.enter_context(tc.tile_pool(name="pt", bufs=3))
        kpool = ctx.enter_context(tc.tile_pool(name="kp", bufs=2))
        wkpool = ctx.enter_context(tc.tile_pool(name="wk", bufs=2))
        qpool = ctx.enter_context(tc.tile_pool(name="qp", bufs=2))
        vpool = ctx.enter_context(tc.tile_pool(name="vp", bufs=2))
        nrm = ctx.enter_context(tc.tile_pool(name="nrm", bufs=2))
        misc = ctx.enter_context(tc.tile_pool(name="misc", bufs=1))

        # ---- persistent loads -------------------------------------------
        # DMA queue order tracks emission order, so the Q-projection's
        # critical inputs (wq, then x's query columns) are emitted first.
        def load_w(which, pool, tag):
            w_sb = pool.tile([P, EC * EMB], f32r, tag=tag, name="w_sb")
            for ec in range(EC):
                nc.sync.dma_start(w_sb[:, ec * EMB:(ec + 1) * EMB],
                                  which[ec * P:(ec + 1) * P, :])
            return w_sb

        bq_sb = misc.tile([P, EC], f32, tag="bq")
        nc.sync.dma_start(bq_sb[:], bqp[:])
        bk_sb = misc.tile([P, EC], f32, tag="bk")
        nc.sync.dma_start(bk_sb[:], bkp[:])
        bob_sb = misc.tile([P, EMB], f32, tag="bob")
        nc.sync.dma_start(bob_sb[:], bob[:])
        wq_sb = load_w(wqt, wpool, "w")
        xt_sb = big.tile([P, EC * SEQ], f32r, tag="xt")
        for ec in range(EC):
            nc.sync.dma_start(xt_sb[:, ec * SEQ: ec * SEQ + QR],
                              xt[ec * P:(ec + 1) * P, 0:QR])
        # wv shares the att slot (disjoint lifetimes); interleave its halves
        # with the x key-column halves in the order the V projection consumes
        wv_sb = big.tile([P, EC * EMB], f32r, tag="att", name="wv_sb")
        for half in range(2):
            for ec in range(EC):
                nc.sync.dma_start(
                    wv_sb[:, ec * EMB + half * NB: ec * EMB + (half + 1) * NB],
                    wvt[ec * P:(ec + 1) * P, half * NB:(half + 1) * NB])
            for ec in range(EC):
                nc.sync.dma_start(
                    xt_sb[:, ec * SEQ + QR + half * NB: ec * SEQ + QR + (half + 1) * NB],
                    xt[ec * P:(ec + 1) * P, QR + half * NB: QR + (half + 1) * NB])
        # ---- Q projection: qstage[oc][p, i] = (x_q @ wq.T + bq).T -------
        for oc in range(EC):
            for ib in range(QR // NB):
                ps = pspool.tile([P, NB], f32, tag="ps")
                for ec in range(EC):
                    nc.tensor.matmul(
                        ps[:],
                        wq_sb[:, ec * EMB + oc * P: ec * EMB + (oc + 1) * P],
                        xt_sb[:, ec * SEQ + ib * NB: ec * SEQ + (ib + 1) * NB],
                        start=(ec == 0), stop=(ec == EC - 1))
                ev = evac.tile([P, NB], f32r, tag="ev")
                nc.vector.tensor_scalar_add(ev[:], ps[:], bq_sb[:, oc:oc + 1])
                nc.gpsimd.dma_start(qstage[oc, :, ib * NB:(ib + 1) * NB], ev[:])

        # wot takes the wq slot; its DMAs overlap the V projection
        wot_sb = load_w(wot, wpool, "w")

        def load_wk(t):
            wk_t = wkpool.tile([P, EC * P], f32r, tag="wk", name="wk_t")
            for ec in range(EC):
                nc.sync.dma_start(
                    wk_t[:, ec * P:(ec + 1) * P],
                    wkt[ec * P:(ec + 1) * P, t * P:(t + 1) * P])
            return wk_t

        wk_next = load_wk(0)

        # ---- V projection (before K so attention can overlap K) ---------
        for ob in range(2):
            for rc in range(RC):
                ps = pspool.tile([P, NB], f32, tag="ps")
                for ec in range(EC):
                    nc.tensor.matmul(
                        ps[:],
                        xt_sb[:, ec * SEQ + rc * P: ec * SEQ + (rc + 1) * P],
                        wv_sb[:, ec * EMB + ob * NB: ec * EMB + (ob + 1) * NB],
                        start=(ec == 0), stop=(ec == EC - 1))
                ev = evac.tile([P, NB], f32r, tag="ev")
                nc.vector.tensor_copy(ev[:], ps[:])
                nc.gpsimd.dma_start(vstage[rc, :, ob * NB:(ob + 1) * NB], ev[:])

        def out_proj(rc8s):
            for rc8 in rc8s:
                for ob in range(2):
                    ps = pspool.tile([P, NB], f32, tag="ps", name="ps")
                    for cc in range(EC):
                        nc.tensor.matmul(
                            ps[:],
                            att_sb[:, cc * QR + rc8 * P: cc * QR + (rc8 + 1) * P],
                            wot_sb[:, cc * EMB + ob * NB: cc * EMB + (ob + 1) * NB],
                            start=(cc == 0), stop=(cc == EC - 1))
                    ev = evac.tile([P, NB], f32, tag="evo", name="ev")
                    nc.vector.tensor_add(
                        ev[:], ps[:], bob_sb[:, ob * NB:(ob + 1) * NB])
                    nc.gpsimd.dma_start(
                        out[rc8 * P:(rc8 + 1) * P, ob * NB:(ob + 1) * NB], ev[:])

        # ---- K projection interleaved with attention ---------------------
        # K o-chunk t == head pair t; it stays in SBUF and feeds attention
        # directly, and the next pair's K matmuls give PE filler work while
        # the current pair's softmax keeps ScalarE busy.  wk is loaded as thin
        # per-pair column slices (the big weight slot is occupied by wot).
        att_sb = big.tile([P, EC * QR], f32r, tag="att")
        for t in range(NH // 2):
            wk_t = wk_next
            if t + 1 < NH // 2:
                wk_next = load_wk(t + 1)
            ktp = kpool.tile([P, SEQ], f32r, tag="kt")
            for jb in range(SEQ // NB):
                ps = pspool.tile([P, NB], f32, tag="ps")
                for ec in range(EC):
                    nc.tensor.matmul(
                        ps[:],
                        wk_t[:, ec * P:(ec + 1) * P],
                        xt_sb[:, ec * SEQ + jb * NB: ec * SEQ + (jb + 1) * NB],
                        start=(ec == 0), stop=(ec == EC - 1))
                nc.vector.tensor_scalar_add(
                    ktp[:, jb * NB:(jb + 1) * NB], ps[:], bk_sb[:, t:t + 1])
            qtp = qpool.tile([P, QR], f32r, tag="qt")
            nc.sync.dma_start(qtp[:], qstage[t, :, :])
            # last pair runs ib-major so half the output projection can start
            # while its second i-block is still in softmax
            ebs = ([(e, ib) for e in range(2) for ib in range(QR // NB)]
                   if t < NH // 2 - 1 else
                   [(e, ib) for ib in range(QR // NB) for e in range(2)])
            vh_tiles = {}
            for e, ib in ebs:
                h = 2 * t + e
                if e not in vh_tiles:
                    vh = vpool.tile([P, RC * 65], f32r, tag="vh", name="vh")
                    vdst = vh[:].rearrange("p (a b) -> p a b", a=RC)
                    vsrc = bass.AP(vstage, h * HD,
                                   [[EMB, P], [P * EMB, RC], [1, HD]])
                    nc.sync.dma_start(vdst[:, :, 0:HD], vsrc)
                    ones_ap = bass.AP(vh.tensor, vh.offset + HD,
                                      [list(vh.ap[0]), [65, RC]])
                    nc.vector.memset(ones_ap.bitcast(f32), 1.0)
                    vh_tiles[e] = vh
                vh = vh_tiles[e]
                if True:
                    ot_ps = otpool.tile([P, NB], f32, tag="ot")
                    for jc2 in range(RC // 2):
                        st_ps = stpool.tile([P, 2 * NB], f32, tag="st")
                        for u in range(2):
                            jc = jc2 * 2 + u
                            nc.tensor.matmul(
                                st_ps[:, u * NB:(u + 1) * NB],
                                ktp[e * HD:(e + 1) * HD,
                                    jc * P:(jc + 1) * P],
                                qtp[e * HD:(e + 1) * HD,
                                    ib * NB:(ib + 1) * NB],
                                start=True, stop=True)
                        pt = ptpool.tile([P, 2 * NB], f32r, tag="pt")
                        nc.scalar.activation(pt[:], st_ps[:], Exp, scale=SCALE)
                        for u in range(2):
                            jc = jc2 * 2 + u
                            nc.tensor.matmul(
                                ot_ps[0:65, :],
                                vh[:, jc * 65:(jc + 1) * 65],
                                pt[:, u * NB:(u + 1) * NB],
                                start=(jc == 0), stop=(jc == RC - 1))
                    # rows 0..63 = head output^T, row 64 = softmax denominator.
                    # Broadcast the denominator row across 64 partitions via a
                    # DRAM bounce (no gpsimd ucode in this toolchain), then a
                    # single fused divide evacuates + normalizes.
                    rs = nrm.tile([P, NB], f32, tag="rs")
                    nc.vector.reciprocal(rs[64:65, :], ot_ps[64:65, :])
                    nc.gpsimd.dma_start(bscr[h, ib, :], rs[64:65, :])
                    bc = nrm.tile([P, NB], f32, tag="bc")
                    bsrc = bass.AP(bscr, (h * 2 + ib) * NB, [[0, HD], [1, NB]])
                    nc.gpsimd.dma_start(bc[0:HD, :], bsrc)
                    nc.vector.tensor_mul(
                        att_sb[e * HD:(e + 1) * HD,
                               t * QR + ib * NB: t * QR + (ib + 1) * NB],
                        ot_ps[0:HD, :], bc[0:HD, :])
                if t == NH // 2 - 1 and e == 1:
                    out_proj(range(ib * (QR // P // 2), (ib + 1) * (QR // P // 2)))

        # ---- output projection (emitted inside out_proj) -----------------

    # gpsimd PartitionBroadcast needs a ucode library selected; mirror
    # Bacc.insert_library_loads on this plain Bass module.
    import bass_rust as _bass_rust
    from concourse.library_config import all_libraries, standard

    inst_type_to_lib_mask = {}
    for lib in all_libraries:
        for inst_type in lib.instructions:
            inst_type_to_lib_mask[inst_type] = inst_type_to_lib_mask.get(
                inst_type, 0) | (1 << lib.index)
    _bass_rust.insert_library_loads(
        nc, inst_type_to_lib_mask, len(all_libraries), standard.index)

    _split_multi_waits(nc, mybir)

    return nc


def _split_multi_waits(nc, mybir):
    """This walrus build accepts at most ONE sync-wait per instruction; Tile
    emits several.  Hoist all but the last wait onto single-wait NoOps placed
    immediately before the instruction on the same engine."""
    nop_id = [0]
    for fn in nc.m.functions:
        for bb in fn.blocks:
            out = []
            for inst in bb.instructions:
                si = inst.sync_info
                if si is not None and si.on_wait is not None \
                        and len(si.on_wait) > 1:
                    waits = list(si.on_wait)
                    for w in waits[:-1]:
                        nop = mybir.InstNoOp(
                            name=f"I-waitsplit-{nop_id[0]}", ins=[], outs=[])
                        nop_id[0] += 1
                        nop.engine = inst.engine
                        nop.sync_info = mybir.SyncInfo(
                            on_wait=[w], on_update=[])
                        out.append(nop)
                    inst.sync_info = mybir.SyncInfo(
                        on_wait=[waits[-1]],
                        on_update=list(si.on_update or []))
                out.append(inst)
            bb.instructions = out


def _get_compiled():
    global _COMPILED
    if _COMPILED is None:
        _COMPILED = _build()
    return _COMPILED


def kernel(x, wq, bq, wk, bk, wv, bv, wo, bo, _want_results_obj=False,
           **run_kwargs):
    from concourse.bass_utils import run_bass_kernel_spmd

    x = np.asarray(x, dtype=np.float32)
    wq = np.asarray(wq, dtype=np.float32)
    bq = np.asarray(bq, dtype=np.float32)
    wk = np.asarray(wk, dtype=np.float32)
    bk = np.asarray(bk, dtype=np.float32)
    wv = np.asarray(wv, dtype=np.float32)
    bv = np.asarray(bv, dtype=np.float32)
    wo = np.asarray(wo, dtype=np.float32)
    bo = np.asarray(bo, dtype=np.float32)

    bs, seq, emb = x.shape
    assert (bs, seq, emb) == (4, SEQ, EMB)

    nc = _get_compiled()

    shared = {
        "wqt": np.ascontiguousarray(wq.T),
        "wkt": np.ascontiguousarray(wk.T),
        "wvt": np.ascontiguousarray(wv.T),
        "wot": np.ascontiguousarray(wo.T),
        "bqp": np.ascontiguousarray(bq.reshape(EC, P).T),
        "bkp": np.ascontiguousarray(bk.reshape(EC, P).T),
        "bob": np.ascontiguousarray(
            np.broadcast_to(bo + wo @ bv, (P, EMB))),
    }
    in_maps = []
    for c in range(8):
        b, hf = c // 2, c % 2
        xb = x[b]
        # this core's query rows first; row order of keys/values is irrelevant
        xb_perm = np.concatenate(
            [xb[hf * QR:(hf + 1) * QR], xb[(1 - hf) * QR:(2 - hf) * QR]], axis=0)
        in_maps.append({
            "xt": np.ascontiguousarray(xb_perm.T),
            **shared,
        })

    res = run_bass_kernel_spmd(nc, in_maps, core_ids=list(range(8)),
                               **run_kwargs)

    outp = np.empty((bs, seq, emb), dtype=np.float32)
    for c in range(8):
        b, hf = c // 2, c % 2
        outp[b, hf * QR:(hf + 1) * QR, :] = res.results[c]["out"]
    if _want_results_obj:
        return outp, res
    return outp



# revision 30
# speedup vs baseline: 1.3201x; 1.3201x over previous
"""Multi-head attention kernel for 8 TRN2 NeuronCores.

Problem: x(4,2048,1024) -> MHA(16 heads, d=64) -> out(4,2048,1024), f32.

Sharding: core c handles (batch b = c//2, seq half = c%2): it computes
attention outputs (incl. all projections) for its 1024 query rows over all 16
heads.  K/V projections for the full batch are computed locally per core (2x
redundant) which keeps cores fully independent - zero collectives.

Precision strategy: projections run in bf16 (same PE cost as f32r here, half
the DMA bytes).  Attention runs in fp8e4m3 with DoubleRow matmuls (two
contraction subtiles per instruction at 0.5 cycles/row => 2x f32r throughput
on both QK^T and PV):
  - scores: stationary K is hi/lo-split across the DR subtiles
    (k = fp8(k) + fp8(k - fp8(k))), moving q is plain fp8 duplicated across
    subtiles -> k's quantization error cancels exactly; only q and p carry
    fp8 error.
  - PV: DR subtiles pair adjacent 128-row k-chunks; v is hi/lo split into two
    accumulation chains into the same PSUM (v enters linearly => exact).  A
    ones column rides at d=64 of the v-hi stationary tile so the softmax
    denominator falls out of the same matmuls (the v-lo chain carries zeros
    there so it is counted once).
End-to-end sim error ~1.7e-2 relative to the f32 reference (gate 2e-2).

Engine plan: ScalarE exp (256 x [128,1024] tiles straight out of PSUM, 1/8
scale and fp8 quantization fused) is the critical engine at ~266us, so the
schedule exists to keep it saturated: a minimal warm-up (Q chunk 0, K pair 0,
V block 0, Q chunk 1) starts the pair-0 attention ASAP, and every remaining
projection psum-group is emitted as PE filler INSIDE the attention jc2 loops
(the PE would otherwise idle waiting on exp between the score and PV matmuls
of consecutive k-chunk pairs).  V evacuates from PSUM directly into the PV
stationary layout in SBUF (scattered per-head writes, zero staging DMA); K
and Q likewise stay in SBUF.  Normalization is reciprocal + a DRAM-bounce
partition-broadcast multiplied during the PSUM->SBUF evacuation.
V-projection bias is folded into the output bias on host (bo_eff = bo+wo@bv).

This walrus build accepts only ONE sync-wait per instruction, so a post-pass
splits multi-wait instructions into single-wait NoOps (_split_multi_waits).
"""

import numpy as np
from contextlib import ExitStack

P = 128
EMB = 1024
SEQ = 2048
QR = 1024          # query rows per core
NH = 16
HD = 64
EC = EMB // P      # 8 contraction chunks
RC = SEQ // P      # 16 seq row chunks
NB = 512           # free-dim block
NP = NH // 2       # 8 head pairs
JC2 = RC // 2      # 8 k-chunk pairs
VW = 2 * HD + 2    # 130: per (head, jc2) stationary v stripe [2, 65]
SCALE = 0.125      # 1/sqrt(64)

_COMPILED = None   # nc cache


def _patch_tile_drain():
    """This walrus build only accepts ONE sync-wait per Drain instruction; the
    stock TileContext tail drain carries one wait per pending proc.  Split it
    into a chain of single-wait drains."""
    import concourse.tile as tile
    from concourse.vector_clock import ScopedClock, VectorClock

    if getattr(tile.TileContext, "_drain_patched", False):
        return

    def _drain_and_barrier(self, tick_clock, wait_clock):
        nc = self.nc
        gc = tick_clock.global_clock
        vals = eval(repr(gc).replace("VectorClock", ""))
        n = len(vals)
        for i, v in enumerate(vals):
            if v > 0:
                sub = VectorClock([vals[j] if j == i else 0 for j in range(n)])
                d = nc.sync.drain()
                wait_clock.add_sem_waits(d.ins, ScopedClock({None: sub}))
        nc.all_engine_barrier()
        popped = nc._tile_sem_poison_stack.pop()
        assert popped is self._sem_poison
        nc.clear_and_free_semaphores(list(self.sems.allocated().values()))
        nc.all_engine_barrier()

    tile.TileContext._drain_and_barrier = _drain_and_barrier
    tile.TileContext._drain_patched = True


def _build():
    import concourse.bass as bass
    import concourse.mybir as mybir
    import concourse.tile as tile

    _patch_tile_drain()

    f32 = mybir.dt.float32
    bf16 = mybir.dt.bfloat16
    fp8 = mybir.dt.float8e4
    Exp = mybir.ActivationFunctionType.Exp
    DR = mybir.MatmulPerfMode.DoubleRow
    Alu = mybir.AluOpType

    nc = bass.Bass()

    # xt holds this core's batch transposed, with the core's 1024 query rows
    # FIRST (host pre-permutes; key/value row order is irrelevant to MHA).
    xt = nc.dram_tensor("xt", [EMB, SEQ], bf16, kind="ExternalInput")
    wqt = nc.dram_tensor("wqt", [EMB, EMB], bf16, kind="ExternalInput")
    wkt = nc.dram_tensor("wkt", [EMB, EMB], bf16, kind="ExternalInput")
    wvt = nc.dram_tensor("wvt", [EMB, EMB], bf16, kind="ExternalInput")
    wot = nc.dram_tensor("wot", [EMB, EMB], bf16, kind="ExternalInput")
    bqp = nc.dram_tensor("bqp", [P, EC], f32, kind="ExternalInput")
    bkp = nc.dram_tensor("bkp", [P, EC], f32, kind="ExternalInput")
    bob = nc.dram_tensor("bob", [P, EMB], f32, kind="ExternalInput")
    out = nc.dram_tensor("out", [QR, EMB], f32, kind="ExternalOutput")

    # softmax denominator bounce buffer (for partition-broadcast via DMA)
    bscr = nc.dram_tensor("bscr", [NH, 2, NB], f32, kind="Internal")


    with tile.TileContext(nc) as tc, ExitStack() as ctx:
        big = ctx.enter_context(tc.tile_pool(name="big", bufs=1))
        wpool = ctx.enter_context(tc.tile_pool(name="w", bufs=1))
        qpool = ctx.enter_context(tc.tile_pool(name="qp", bufs=1))
        vbpool = ctx.enter_context(tc.tile_pool(name="vb", bufs=1))
        pspool = ctx.enter_context(tc.tile_pool(name="ps", bufs=2, space="PSUM"))
        stpool = ctx.enter_context(tc.tile_pool(name="st", bufs=2, space="PSUM"))
        otpool = ctx.enter_context(tc.tile_pool(name="ot", bufs=2, space="PSUM"))
        evac = ctx.enter_context(tc.tile_pool(name="evac", bufs=3))
        ptpool = ctx.enter_context(tc.tile_pool(name="pt", bufs=3))
        kpool = ctx.enter_context(tc.tile_pool(name="kp", bufs=3))
        wkpool = ctx.enter_context(tc.tile_pool(name="wk", bufs=3))
        nrm = ctx.enter_context(tc.tile_pool(name="nrm", bufs=1))
        misc = ctx.enter_context(tc.tile_pool(name="misc", bufs=1))

        # ---- persistent loads -------------------------------------------
        # DMA queue order tracks emission order; pair-0's critical inputs
        # (wq, x query columns, wk0) go first so ScalarE exp starts early.
        bq_sb = misc.tile([P, EC], f32, tag="bq")
        nc.sync.dma_start(bq_sb[:], bqp[:])
        bk_sb = misc.tile([P, EC], f32, tag="bk")
        nc.sync.dma_start(bk_sb[:], bkp[:])

        # Loads are column-sliced by CONSUMER (one multi-dim-AP DMA per
        # slice): the first Q/K/score work needs only wq columns oc=0, the
        # first 512 x columns, and wk pair 0 -- ~4us of DMA instead of the
        # full 12MB.  One DMA instruction per slice also matters: the SP
        # sequencer serializes DMA issue at ~650ns each.
        wq_sb = wpool.tile([P, EC * EMB], bf16, tag="w", name="wq_sb")
        wq3 = wq_sb[:].rearrange("p (ec n) -> p ec n", ec=EC)

        def load_wq_oc(oc):  # loads the oc-PAIR oc, oc+1 (1KB elem runs)
            nc.sync.dma_start(
                wq3[:, :, oc * P:(oc + 2) * P],
                bass.AP(wqt, oc * P, [[EMB, P], [P * EMB, EC], [1, 2 * P]]))

        xt_sb = big.tile([P, EC * SEQ], bf16, tag="xt")
        xt3 = xt_sb[:].rearrange("p (ec n) -> p ec n", ec=EC)

        def load_x_cols(c0, n):
            nc.sync.dma_start(
                xt3[:, :, c0:c0 + n],
                bass.AP(xt, c0, [[SEQ, P], [P * SEQ, EC], [1, n]]))

        def load_wk(t):
            wk_t = wkpool.tile([P, EC * P], bf16, tag="wk", name="wk_t")
            nc.sync.dma_start(
                wk_t[:].rearrange("p (ec n) -> p ec n", ec=EC),
                bass.AP(wkt, t * P, [[EMB, P], [P * EMB, EC], [1, P]]))
            return wk_t

        wv_sb = big.tile([P, EC * EMB], bf16, tag="wv", name="wv_sb")
        wv3 = wv_sb[:].rearrange("p (ec n) -> p ec n", ec=EC)

        def load_wv(ob):
            nc.sync.dma_start(
                wv3[:, :, ob * 256:(ob + 1) * 256],
                bass.AP(wvt, ob * 256, [[EMB, P], [P * EMB, EC], [1, 256]]))

        # criticality order: pair-0 scores first, then key columns (in two
        # halves so K-proj jb2 starts at the halfway point), then the rest
        load_wq_oc(0)
        load_x_cols(0, NB)              # x query cols, first half
        wk_tiles = {0: load_wk(0)}
        load_x_cols(NB, NB)             # x query cols, second half
        load_wv(0)
        load_x_cols(QR, NB)             # x key cols, first half
        load_x_cols(QR + NB, NB)        # x key cols, second half
        for oc in range(2, EC, 2):
            load_wq_oc(oc)
        for ob in range(1, 4):
            load_wv(ob)

        bob_sb = misc.tile([P, EMB], f32, tag="bob")
        nc.sync.dma_start(bob_sb[:], bob[:])
        # wot gets its OWN slab (sharing wq's would chain its DMA behind
        # every Q-projection read AND block later SP-queue DMAs on that wait)
        wot_sb = wpool.tile([P, EC * EMB], bf16, tag="wo", name="wot_sb")
        nc.sync.dma_start(
            wot_sb[:].rearrange("p (ec n) -> p ec n", ec=EC),
            bass.AP(wot, 0, [[EMB, P], [P * EMB, EC], [1, EMB]]))

        # ---- persistent SBUF state --------------------------------------
        # q: per pair t, fp8 [128, 2(dup), 1024].  (A stride-0 moving dim
        # would avoid the duplication but miscomputes on HW for 64-partition
        # operands; duplicated real-stride subtiles are the verified shape.)
        qall = qpool.tile([P, NP * 2 * QR], fp8, tag="qall")
        # v in PV-stationary layout: one 193B-pitch stripe per
        # (head, k-chunk): [v_hi(64) | ones | lo@+128: v_lo(64) | zero].
        # The DR subtile pair is (hi, lo) at stride 128 / width 128 (the
        # only shape the s3_lw_dual_fp8 ISA check accepts); the width-128
        # read overspills 63 bytes into the NEXT stripe's (written, non-NaN)
        # data, multiplying into PSUM rows 65..127 which are never read.
        # Row 64 of the PSUM output is the softmax denominator (ones lane).
        VP_ = 193
        vsb = vbpool.tile([P, NH * RC * VP_ + 64], fp8, tag="vsb")
        # ones lane at col 64; cols 65..127 (the gap before lo) must also be
        # finite -- fill with ones too (they land in unread PSUM rows).
        # Per-head-block so pair 0 isn't gated on the whole sweep.
        for ob_ in range(4):
            base_ = vsb.offset + 4 * ob_ * RC * VP_
            nc.gpsimd.memset(
                bass.AP(vsb.tensor, base_ + HD,
                        [list(vsb.ap[0]), [VP_, 4 * RC], [1, HD]]), 1.0)
            nc.gpsimd.memset(
                bass.AP(vsb.tensor, base_ + 192,
                        [list(vsb.ap[0]), [VP_, 4 * RC]]), 0.0)
        nc.vector.memset(vsb[:, NH * RC * VP_:], 0.0)  # tail pad

        def v_stat(h, jc):
            return bass.AP(vsb.tensor, vsb.offset + (h * RC + jc) * VP_,
                           [list(vsb.ap[0]), [128, 2], [1, 128]])
        att_sb = big.tile([P, EC * QR], bf16, tag="att")

        # ---- projection psum-groups (also the PE filler units) ----------
        def q_group(oc, ib):
            ps = pspool.tile([P, NB], f32, tag="ps", name="ps")
            for ec in range(EC):
                nc.tensor.matmul(
                    ps[:],
                    wq_sb[:, ec * EMB + oc * P: ec * EMB + (oc + 1) * P],
                    xt_sb[:, ec * SEQ + ib * NB: ec * SEQ + (ib + 1) * NB],
                    start=(ec == 0), stop=(ec == EC - 1))
            for u in range(2):
                nc.vector.tensor_scalar_add(
                    qall[:, oc * 2 * QR + u * QR + ib * NB:
                         oc * 2 * QR + u * QR + (ib + 1) * NB],
                    ps[:], bq_sb[:, oc:oc + 1])

        def v_group(ob, rc):
            # 256 v columns (heads 4ob..4ob+3) for seq chunk rc, written
            # straight into the PV stationary layout
            psf = pspool.tile([P, NB], f32, tag="ps", name="ps")
            ps = psf[:, 0:256]
            for ec in range(EC):
                nc.tensor.matmul(
                    ps,
                    xt_sb[:, ec * SEQ + rc * P: ec * SEQ + (rc + 1) * P],
                    wv_sb[:, ec * EMB + ob * 256: ec * EMB + (ob + 1) * 256],
                    start=(ec == 0), stop=(ec == EC - 1))
            def scat(half):
                return bass.AP(
                    vsb.tensor,
                    vsb.offset + (4 * ob * RC + rc) * VP_ + half * 128,
                    [list(vsb.ap[0]), [RC * VP_, 4], [1, HD]])
            hi = scat(0)
            nc.vector.tensor_copy(hi, ps)
            nc.vector.tensor_sub(scat(1), ps, hi)

        ktp_tiles = {}

        def k_group(t, jb):
            # k stored per 128-kpos chunk as [hi(128) | lo(128)] so the DR
            # stationary is the verified stride-128/width-128 shape
            if t not in ktp_tiles:
                if t not in wk_tiles:
                    wk_tiles[t] = load_wk(t)
                ktp_tiles[t] = kpool.tile([P, 2 * SEQ], fp8, tag="kt",
                                          name="ktp")
            wk_t = wk_tiles[t]
            ktp = ktp_tiles[t]
            ps = pspool.tile([P, NB], f32, tag="ps", name="ps")
            for ec in range(EC):
                nc.tensor.matmul(
                    ps[:],
                    wk_t[:, ec * P:(ec + 1) * P],
                    xt_sb[:, ec * SEQ + jb * NB: ec * SEQ + (jb + 1) * NB],
                    start=(ec == 0), stop=(ec == EC - 1))

            def kscat(half):
                return bass.AP(ktp.tensor,
                               ktp.offset + jb * 4 * 256 + half * 128,
                               [list(ktp.ap[0]), [256, 4], [1, 128]])
            hi = kscat(0)
            nc.vector.tensor_scalar_add(hi, ps[:], bk_sb[:, t:t + 1])
            nc.vector.scalar_tensor_tensor(
                kscat(1), ps[:], bk_sb[:, t:t + 1], hi,
                op0=Alu.add, op1=Alu.subtract)

        def o_group(rc8, ob):
            ps = pspool.tile([P, NB], f32, tag="ps", name="ps")
            for cc in range(EC):
                nc.tensor.matmul(
                    ps[:],
                    att_sb[:, cc * QR + rc8 * P: cc * QR + (rc8 + 1) * P],
                    wot_sb[:, cc * EMB + ob * NB: cc * EMB + (ob + 1) * NB],
                    start=(cc == 0), stop=(cc == EC - 1))
            ev = evac.tile([P, NB], f32, tag="evo", name="ev")
            nc.vector.tensor_add(ev[:], ps[:], bob_sb[:, ob * NB:(ob + 1) * NB])
            # HWDGE, not gpsimd: the Pool/Q7 descriptor gen (~1us each) must
            # stay free for the normalize bounce on the critical tail
            nc.sync.dma_start(
                out[rc8 * P:(rc8 + 1) * P, ob * NB:(ob + 1) * NB], ev[:])

        # ---- warm-up: pair-0 critical path ------------------------------
        q_group(0, 0)
        k_group(0, 0)
        q_group(0, 1)
        k_group(0, 1)
        for rc in range(8):       # seq chunks 0..7 are the query half: no
            v_group(0, rc)        # dependence on the (later) key-column DMAs
        for ib in range(2):
            q_group(1, ib)

        # ---- filler queue (ordered; (needed_by_pair, emit_fn)) ----------
        fill = []

        def add_kq(t):
            fill.extend((t, lambda t=t, jb=jb: k_group(t, jb))
                        for jb in range(4))
            fill.extend((t, lambda t=t, ib=ib: q_group(t, ib))
                        for ib in range(2))

        def add_v(ob):
            fill.extend((max(2 * ob, 1), lambda ob=ob, rc=rc: v_group(ob, rc))
                        for rc in range(RC))

        fill.extend((1, lambda jb=jb: k_group(0, jb)) for jb in (2, 3))
        fill.extend((1, lambda rc=rc: v_group(0, rc)) for rc in range(8, RC))
        fill.extend((1, lambda jb=jb: k_group(1, jb)) for jb in range(4))
        add_kq(2)
        add_v(1)
        add_kq(3)
        add_v(2)
        add_kq(4)
        add_kq(5)
        add_v(3)
        add_kq(6)
        add_kq(7)
        fill_i = [0]

        def ensure_ready(t):
            while fill_i[0] < len(fill) and fill[fill_i[0]][0] <= t:
                fill[fill_i[0]][1]()
                fill_i[0] += 1

        def pull_filler():
            if fill_i[0] < len(fill):
                fill[fill_i[0]][1]()
                fill_i[0] += 1

        # ---- attention: one software-pipelined stream -------------------
        # PV runs one jc2 step behind exp so the PE never sits on the exp
        # latency; the stream crosses (e, ib) and pair boundaries without a
        # sync point (the next block's scores are emitted before the
        # previous block's last PV + normalize).
        steps = []
        for t in range(NP):
            # last pair runs ib-major so half the output projection can
            # start while its second i-block is still in softmax
            ebs = ([(e, ib) for e in range(2) for ib in range(QR // NB)]
                   if t < NP - 1 else
                   [(e, ib) for ib in range(QR // NB) for e in range(2)])
            steps += [(t, e, ib, jc2) for e, ib in ebs for jc2 in range(JC2)]

        kt3 = qv = None
        cur_t = -1
        blocks = {}   # (t, e, ib) -> ot_ps
        pend = None   # (t, e, ib, jc2, pt3)

        def finish_block(t, e, ib):
            # rows 0..63 = head output^T, row 64 = softmax denominator.
            # Broadcast the denominator row across 64 partitions via a DRAM
            # bounce, then a fused multiply evacuates + normalizes.
            h = 2 * t + e
            eb = e * HD
            ot_ps = blocks.pop((t, e, ib))
            rs = nrm.tile([P, NB], f32, tag="rs")
            nc.vector.reciprocal(rs[64:65, :], ot_ps[64:65, :])
            nc.gpsimd.dma_start(bscr[h, ib, :], rs[64:65, :])
            bc = nrm.tile([P, NB], f32, tag="bc")
            bsrc = bass.AP(bscr, (h * 2 + ib) * NB, [[0, HD], [1, NB]])
            nc.gpsimd.dma_start(bc[0:HD, :], bsrc)
            nc.vector.tensor_mul(
                att_sb[eb:eb + HD, t * QR + ib * NB: t * QR + (ib + 1) * NB],
                ot_ps[0:HD, :], bc[0:HD, :])
            if t == NP - 1 and e == 1:
                for rc8 in range(ib * 4, ib * 4 + 4):
                    for ob in range(2):
                        o_group(rc8, ob)

        def emit_pv(p):
            t, e, ib, jc2, pt = p
            for u in range(2):
                jc = jc2 * 2 + u
                ptu = bass.AP(pt.tensor, pt.offset + u * NB,
                              [list(pt.ap[0]), [0, 2], [1, NB]])
                nc.tensor.matmul(
                    blocks[(t, e, ib)][:, :], v_stat(2 * t + e, jc), ptu,
                    start=(jc == 0), stop=(jc == RC - 1),
                    perf_mode=DR)

        for t, e, ib, jc2 in steps:
            if t != cur_t:
                ensure_ready(t)
                ktp_t = ktp_tiles[t]
                qv = qall[:, t * 2 * QR:(t + 1) * 2 * QR] \
                    .rearrange("p (u n) -> p u n", u=2)
                cur_t = t
            if jc2 == 0:
                ot_new = otpool.tile([P, NB], f32, tag="ot", name="ot_new")
                blocks[(t, e, ib)] = ot_new
            eb = e * HD
            st_ps = stpool.tile([P, 2 * NB], f32, tag="st")
            for u in range(2):
                jc = jc2 * 2 + u
                ksl = ktp_t[eb:eb + HD, jc * 256:(jc + 1) * 256]
                nc.tensor.matmul(
                    st_ps[:, u * NB:(u + 1) * NB],
                    bass.AP(ksl.tensor, ksl.offset,
                            [list(ksl.ap[0]), [128, 2], [1, 128]]),
                    qv[eb:eb + HD, :, ib * NB:(ib + 1) * NB],
                    start=True, stop=True, perf_mode=DR)
            pt = ptpool.tile([P, 2 * NB], fp8, tag="pt")
            nc.scalar.activation(pt[:], st_ps[:], Exp, scale=SCALE)
            # filler BEFORE the pending PV: a pulled v_group must land ahead
            # of the first PV that consumes it
            if (t, e, ib) == (0, 0, 0):
                if jc2 < 5:
                    pull_filler()
                    pull_filler()
            elif jc2 % 2 == 1:
                pull_filler()
            if pend is not None:
                emit_pv(pend)
                if pend[3] == JC2 - 1:
                    finish_block(*pend[:3])
            pend = (t, e, ib, jc2, pt)
        emit_pv(pend)
        finish_block(*pend[:3])

    # gpsimd SWDGE queues need a ucode library selected; mirror
    # Bacc.insert_library_loads on this plain Bass module.
    import bass_rust as _bass_rust
    from concourse.library_config import all_libraries, standard

    inst_type_to_lib_mask = {}
    for lib in all_libraries:
        for inst_type in lib.instructions:
            inst_type_to_lib_mask[inst_type] = inst_type_to_lib_mask.get(
                inst_type, 0) | (1 << lib.index)
    _bass_rust.insert_library_loads(
        nc, inst_type_to_lib_mask, len(all_libraries), standard.index)

    _split_multi_waits(nc, mybir)

    return nc


def _split_multi_waits(nc, mybir):
    """This walrus build accepts at most ONE sync-wait per instruction; Tile
    emits several.  Hoist all but the last wait onto single-wait NoOps placed
    immediately before the instruction on the same engine."""
    nop_id = [0]
    for fn in nc.m.functions:
        for bb in fn.blocks:
            out = []
            for inst in bb.instructions:
                si = inst.sync_info
                if si is not None and si.on_wait is not None \
                        and len(si.on_wait) > 1:
                    waits = list(si.on_wait)
                    for w in waits[:-1]:
                        nop = mybir.InstNoOp(
                            name=f"I-waitsplit-{nop_id[0]}", ins=[], outs=[])
                        nop_id[0] += 1
                        nop.engine = inst.engine
                        nop.sync_info = mybir.SyncInfo(
                            on_wait=[w], on_update=[])
                        out.append(nop)
                    inst.sync_info = mybir.SyncInfo(
                        on_wait=[waits[-1]],
                        on_update=list(si.on_update or []))
                out.append(inst)
            bb.instructions = out


def _get_compiled():
    global _COMPILED
    if _COMPILED is None:
        _COMPILED = _build()
    return _COMPILED


def kernel(x, wq, bq, wk, bk, wv, bv, wo, bo, _want_results_obj=False,
           **run_kwargs):
    import ml_dtypes
    from concourse.bass_utils import run_bass_kernel_spmd

    bf = ml_dtypes.bfloat16
    x = np.asarray(x, dtype=np.float32)
    wq = np.asarray(wq, dtype=np.float32)
    bq = np.asarray(bq, dtype=np.float32)
    wk = np.asarray(wk, dtype=np.float32)
    bk = np.asarray(bk, dtype=np.float32)
    wv = np.asarray(wv, dtype=np.float32)
    bv = np.asarray(bv, dtype=np.float32)
    wo = np.asarray(wo, dtype=np.float32)
    bo = np.asarray(bo, dtype=np.float32)

    bs, seq, emb = x.shape
    assert (bs, seq, emb) == (4, SEQ, EMB)

    nc = _get_compiled()

    shared = {
        "wqt": np.ascontiguousarray(wq.T).astype(bf),
        "wkt": np.ascontiguousarray(wk.T).astype(bf),
        "wvt": np.ascontiguousarray(wv.T).astype(bf),
        "wot": np.ascontiguousarray(wo.T).astype(bf),
        "bqp": np.ascontiguousarray(bq.reshape(EC, P).T),
        "bkp": np.ascontiguousarray(bk.reshape(EC, P).T),
        "bob": np.ascontiguousarray(
            np.broadcast_to(bo + wo @ bv, (P, EMB))),
    }
    in_maps = []
    for c in range(8):
        b, hf = c // 2, c % 2
        xb = x[b]
        # this core's query rows first; row order of keys/values is irrelevant
        xb_perm = np.concatenate(
            [xb[hf * QR:(hf + 1) * QR], xb[(1 - hf) * QR:(2 - hf) * QR]], axis=0)
        in_maps.append({
            "xt": np.ascontiguousarray(xb_perm.T).astype(bf),
            **shared,
        })

    res = run_bass_kernel_spmd(nc, in_maps, core_ids=list(range(8)),
                               **run_kwargs)

    outp = np.empty((bs, seq, emb), dtype=np.float32)
    for c in range(8):
        b, hf = c // 2, c % 2
        outp[b, hf * QR:(hf + 1) * QR, :] = res.results[c]["out"]
    if _want_results_obj:
        return outp, res
    return outp


# revision 42
# speedup vs baseline: 1.3353x; 1.0115x over previous
"""Multi-head attention kernel for 8 TRN2 NeuronCores.

Problem: x(4,2048,1024) -> MHA(16 heads, d=64) -> out(4,2048,1024), f32.

Sharding: core c handles (batch b = c//2, seq half = c%2): it computes
attention outputs (incl. all projections) for its 1024 query rows over all 16
heads.  K/V projections for the full batch are computed locally per core (2x
redundant) which keeps cores fully independent - zero collectives.

Precision strategy: projections run in bf16 (same PE cost as f32r here, half
the DMA bytes).  Attention runs in fp8e4m3 with DoubleRow matmuls (two
contraction subtiles per instruction at 0.5 cycles/row => 2x f32r throughput
on both QK^T and PV):
  - scores: stationary K is hi/lo-split across the DR subtiles
    (k = fp8(k) + fp8(k - fp8(k))), moving q is plain fp8 duplicated across
    subtiles -> k's quantization error cancels exactly; only q and p carry
    fp8 error.
  - PV: DR subtiles pair adjacent 128-row k-chunks; v is hi/lo split into two
    accumulation chains into the same PSUM (v enters linearly => exact).  A
    ones column rides at d=64 of the v-hi stationary tile so the softmax
    denominator falls out of the same matmuls (the v-lo chain carries zeros
    there so it is counted once).
End-to-end sim error ~1.7e-2 relative to the f32 reference (gate 2e-2).

Engine plan: ScalarE exp (256 x [128,1024] tiles straight out of PSUM, 1/8
scale and fp8 quantization fused) is the critical engine at ~266us, so the
schedule exists to keep it saturated: a minimal warm-up (Q chunk 0, K pair 0,
V block 0, Q chunk 1) starts the pair-0 attention ASAP, and every remaining
projection psum-group is emitted as PE filler INSIDE the attention jc2 loops
(the PE would otherwise idle waiting on exp between the score and PV matmuls
of consecutive k-chunk pairs).  V evacuates from PSUM directly into the PV
stationary layout in SBUF (scattered per-head writes, zero staging DMA); K
and Q likewise stay in SBUF.  Normalization is reciprocal + a DRAM-bounce
partition-broadcast multiplied during the PSUM->SBUF evacuation.
V-projection bias is folded into the output bias on host (bo_eff = bo+wo@bv).

This walrus build accepts only ONE sync-wait per instruction, so a post-pass
splits multi-wait instructions into single-wait NoOps (_split_multi_waits).
"""

import numpy as np
from contextlib import ExitStack

P = 128
EMB = 1024
SEQ = 2048
QR = 1024          # query rows per core
NH = 16
HD = 64
EC = EMB // P      # 8 contraction chunks
RC = SEQ // P      # 16 seq row chunks
NB = 512           # free-dim block
NP = NH // 2       # 8 head pairs
JC2 = RC // 2      # 8 k-chunk pairs
VW = 2 * HD + 2    # 130: per (head, jc2) stationary v stripe [2, 65]
SCALE = 0.125      # 1/sqrt(64)

_COMPILED = None   # nc cache


def _patch_tile_drain():
    """This walrus build only accepts ONE sync-wait per Drain instruction; the
    stock TileContext tail drain carries one wait per pending proc.  Split it
    into a chain of single-wait drains."""
    import concourse.tile as tile
    from concourse.vector_clock import ScopedClock, VectorClock

    if getattr(tile.TileContext, "_drain_patched", False):
        return

    def _drain_and_barrier(self, tick_clock, wait_clock):
        nc = self.nc
        gc = tick_clock.global_clock
        vals = eval(repr(gc).replace("VectorClock", ""))
        n = len(vals)
        for i, v in enumerate(vals):
            if v > 0:
                sub = VectorClock([vals[j] if j == i else 0 for j in range(n)])
                d = nc.sync.drain()
                wait_clock.add_sem_waits(d.ins, ScopedClock({None: sub}))
        nc.all_engine_barrier()
        popped = nc._tile_sem_poison_stack.pop()
        assert popped is self._sem_poison
        nc.clear_and_free_semaphores(list(self.sems.allocated().values()))
        nc.all_engine_barrier()

    tile.TileContext._drain_and_barrier = _drain_and_barrier
    tile.TileContext._drain_patched = True


def _build():
    import concourse.bass as bass
    import concourse.mybir as mybir
    import concourse.tile as tile

    _patch_tile_drain()

    f32 = mybir.dt.float32
    bf16 = mybir.dt.bfloat16
    fp8 = mybir.dt.float8e4
    Exp = mybir.ActivationFunctionType.Exp
    DR = mybir.MatmulPerfMode.DoubleRow
    Alu = mybir.AluOpType

    nc = bass.Bass()

    # xt holds this core's batch transposed, with the core's 1024 query rows
    # FIRST (host pre-permutes; key/value row order is irrelevant to MHA).
    xt = nc.dram_tensor("xt", [EMB, SEQ], bf16, kind="ExternalInput")
    wqt = nc.dram_tensor("wqt", [EMB, EMB], bf16, kind="ExternalInput")
    wkt = nc.dram_tensor("wkt", [EMB, EMB], bf16, kind="ExternalInput")
    wvt = nc.dram_tensor("wvt", [EMB, EMB], bf16, kind="ExternalInput")
    wot = nc.dram_tensor("wot", [EMB, EMB], bf16, kind="ExternalInput")
    bqp = nc.dram_tensor("bqp", [P, EC], f32, kind="ExternalInput")
    bkp = nc.dram_tensor("bkp", [P, EC], f32, kind="ExternalInput")
    bob = nc.dram_tensor("bob", [P, EMB], f32, kind="ExternalInput")
    out = nc.dram_tensor("out", [QR, EMB], f32, kind="ExternalOutput")

    # softmax denominator bounce buffer (for partition-broadcast via DMA)
    bscr = nc.dram_tensor("bscr", [NH, 2, NB], f32, kind="Internal")


    with tile.TileContext(nc) as tc, ExitStack() as ctx:
        big = ctx.enter_context(tc.tile_pool(name="big", bufs=1))
        wpool = ctx.enter_context(tc.tile_pool(name="w", bufs=1))
        qpool = ctx.enter_context(tc.tile_pool(name="qp", bufs=1))
        vbpool = ctx.enter_context(tc.tile_pool(name="vb", bufs=1))
        pspool = ctx.enter_context(tc.tile_pool(name="ps", bufs=2, space="PSUM"))
        stpool = ctx.enter_context(tc.tile_pool(name="st", bufs=2, space="PSUM"))
        otpool = ctx.enter_context(tc.tile_pool(name="ot", bufs=2, space="PSUM"))
        evac = ctx.enter_context(tc.tile_pool(name="evac", bufs=4))
        ptpool = ctx.enter_context(tc.tile_pool(name="pt", bufs=3))
        kpool = ctx.enter_context(tc.tile_pool(name="kp", bufs=3))
        wkpool = ctx.enter_context(tc.tile_pool(name="wk", bufs=3))
        nrm = ctx.enter_context(tc.tile_pool(name="nrm", bufs=1))
        misc = ctx.enter_context(tc.tile_pool(name="misc", bufs=1))

        # ---- persistent loads -------------------------------------------
        # DMA queue order tracks emission order; pair-0's critical inputs
        # (wq, x query columns, wk0) go first so ScalarE exp starts early.

        # Loads are column-sliced by CONSUMER (one multi-dim-AP DMA per
        # slice): the first Q/K/score work needs only wq columns oc=0, the
        # first 512 x columns, and wk pair 0 -- ~4us of DMA instead of the
        # full 12MB.  One DMA instruction per slice also matters: the SP
        # sequencer serializes DMA issue at ~650ns each.
        wq_sb = wpool.tile([P, EC * EMB], bf16, tag="w", name="wq_sb")
        wq3 = wq_sb[:].rearrange("p (ec n) -> p ec n", ec=EC)

        def load_wq_oc(oc):  # loads the oc-PAIR oc, oc+1 (1KB elem runs)
            nc.sync.dma_start(
                wq3[:, :, oc * P:(oc + 2) * P],
                bass.AP(wqt, oc * P, [[EMB, P], [P * EMB, EC], [1, 2 * P]]))

        xt_sb = big.tile([P, EC * SEQ], bf16, tag="xt")
        xt3 = xt_sb[:].rearrange("p (ec n) -> p ec n", ec=EC)

        def load_x_cols(c0, n):
            nc.sync.dma_start(
                xt3[:, :, c0:c0 + n],
                bass.AP(xt, c0, [[SEQ, P], [P * SEQ, EC], [1, n]]))

        def load_wk(t):
            wk_t = wkpool.tile([P, EC * P], bf16, tag="wk", name="wk_t")
            nc.sync.dma_start(
                wk_t[:].rearrange("p (ec n) -> p ec n", ec=EC),
                bass.AP(wkt, t * P, [[EMB, P], [P * EMB, EC], [1, P]]))
            return wk_t

        wv_sb = big.tile([P, EC * EMB], bf16, tag="wv", name="wv_sb")
        wv3 = wv_sb[:].rearrange("p (ec n) -> p ec n", ec=EC)

        def load_wv(ob):
            nc.sync.dma_start(
                wv3[:, :, ob * 256:(ob + 1) * 256],
                bass.AP(wvt, ob * 256, [[EMB, P], [P * EMB, EC], [1, 256]]))

        # criticality order: pair-0 scores first, then key columns (in two
        # halves so K-proj jb2 starts at the halfway point), then the rest
        load_wq_oc(0)
        load_x_cols(0, NB)              # x query cols, first half
        wk_tiles = {0: load_wk(0)}
        load_x_cols(NB, NB)             # x query cols, second half
        load_wv(0)
        bq_sb = misc.tile([P, EC], f32, tag="bq")
        nc.sync.dma_start(bq_sb[:], bqp[:])
        bk_sb = misc.tile([P, EC], f32, tag="bk")
        nc.sync.dma_start(bk_sb[:], bkp[:])
        load_x_cols(QR, NB)             # x key cols, first half
        load_x_cols(QR + NB, NB)        # x key cols, second half
        wk_tiles[1] = load_wk(1)
        for oc in range(2, EC, 2):
            load_wq_oc(oc)
        for ob in range(1, 4):
            load_wv(ob)

        bob_sb = misc.tile([P, EMB], f32, tag="bob")
        nc.sync.dma_start(bob_sb[:], bob[:])
        # wot gets its OWN slab (sharing wq's would chain its DMA behind
        # every Q-projection read AND block later SP-queue DMAs on that
        # wait); the DMA itself is deferred into the filler queue so it
        # doesn't sit ahead of early wk loads on the DMA engines
        wot_sb = wpool.tile([P, EC * EMB], bf16, tag="wo", name="wot_sb")

        def load_wot():
            nc.sync.dma_start(
                wot_sb[:].rearrange("p (ec n) -> p ec n", ec=EC),
                bass.AP(wot, 0, [[EMB, P], [P * EMB, EC], [1, EMB]]))

        # ---- persistent SBUF state --------------------------------------
        # q: per pair t, fp8 [128, 2(dup), 1024].  (A stride-0 moving dim
        # would avoid the duplication but miscomputes on HW for 64-partition
        # operands; duplicated real-stride subtiles are the verified shape.)
        qall = qpool.tile([P, NP * 2 * QR], fp8, tag="qall")
        # v in PV-stationary layout: one 193B-pitch stripe per
        # (head, k-chunk): [v_hi(64) | ones | lo@+128: v_lo(64) | zero].
        # The DR subtile pair is (hi, lo) at stride 128 / width 128 (the
        # only shape the s3_lw_dual_fp8 ISA check accepts); the width-128
        # read overspills 63 bytes into the NEXT stripe's (written, non-NaN)
        # data, multiplying into PSUM rows 65..127 which are never read.
        # Row 64 of the PSUM output is the softmax denominator (ones lane).
        VP_ = 193
        vsb = vbpool.tile([P, NH * RC * VP_ + 64], fp8, tag="vsb")
        # ones lane at col 64; cols 65..127 (the gap before lo) must also be
        # finite -- fill with ones too (they land in unread PSUM rows).
        # Per-head-block so pair 0 isn't gated on the whole sweep.
        for ob_ in range(4):
            base_ = vsb.offset + 4 * ob_ * RC * VP_
            nc.gpsimd.memset(
                bass.AP(vsb.tensor, base_ + HD,
                        [list(vsb.ap[0]), [VP_, 4 * RC], [1, HD]]), 1.0)
            nc.gpsimd.memset(
                bass.AP(vsb.tensor, base_ + 192,
                        [list(vsb.ap[0]), [VP_, 4 * RC]]), 0.0)
        nc.vector.memset(vsb[:, NH * RC * VP_:], 0.0)  # tail pad

        def v_stat(h, jc):
            return bass.AP(vsb.tensor, vsb.offset + (h * RC + jc) * VP_,
                           [list(vsb.ap[0]), [128, 2], [1, 128]])
        att_sb = big.tile([P, EC * QR], bf16, tag="att")

        # ---- projection psum-groups (also the PE filler units) ----------
        def q_group(oc, ib):
            ps = pspool.tile([P, NB], f32, tag="ps", name="ps")
            for ec in range(EC):
                nc.tensor.matmul(
                    ps[:],
                    wq_sb[:, ec * EMB + oc * P: ec * EMB + (oc + 1) * P],
                    xt_sb[:, ec * SEQ + ib * NB: ec * SEQ + (ib + 1) * NB],
                    start=(ec == 0), stop=(ec == EC - 1))
            for u in range(2):
                nc.vector.tensor_scalar_add(
                    qall[:, oc * 2 * QR + u * QR + ib * NB:
                         oc * 2 * QR + u * QR + (ib + 1) * NB],
                    ps[:], bq_sb[:, oc:oc + 1])

        def v_group(ob, rc):
            # 256 v columns (heads 4ob..4ob+3) for seq chunk rc, written
            # straight into the PV stationary layout
            psf = pspool.tile([P, NB], f32, tag="ps", name="ps")
            ps = psf[:, 0:256]
            for ec in range(EC):
                nc.tensor.matmul(
                    ps,
                    xt_sb[:, ec * SEQ + rc * P: ec * SEQ + (rc + 1) * P],
                    wv_sb[:, ec * EMB + ob * 256: ec * EMB + (ob + 1) * 256],
                    start=(ec == 0), stop=(ec == EC - 1))
            def scat(half):
                return bass.AP(
                    vsb.tensor,
                    vsb.offset + (4 * ob * RC + rc) * VP_ + half * 128,
                    [list(vsb.ap[0]), [RC * VP_, 4], [1, HD]])
            hi = scat(0)
            nc.vector.tensor_copy(hi, ps)
            nc.vector.tensor_sub(scat(1), ps, hi)

        ktp_tiles = {}

        def k_group(t, jb):
            # k stored per 128-kpos chunk as [hi(128) | lo(128)] so the DR
            # stationary is the verified stride-128/width-128 shape
            if t not in ktp_tiles:
                if t not in wk_tiles:
                    wk_tiles[t] = load_wk(t)
                ktp_tiles[t] = kpool.tile([P, 2 * SEQ], fp8, tag="kt",
                                          name="ktp")
            wk_t = wk_tiles[t]
            ktp = ktp_tiles[t]
            ps = pspool.tile([P, NB], f32, tag="ps", name="ps")
            for ec in range(EC):
                nc.tensor.matmul(
                    ps[:],
                    wk_t[:, ec * P:(ec + 1) * P],
                    xt_sb[:, ec * SEQ + jb * NB: ec * SEQ + (jb + 1) * NB],
                    start=(ec == 0), stop=(ec == EC - 1))

            def kscat(half):
                return bass.AP(ktp.tensor,
                               ktp.offset + jb * 4 * 256 + half * 128,
                               [list(ktp.ap[0]), [256, 4], [1, 128]])
            hi = kscat(0)
            nc.vector.tensor_scalar_add(hi, ps[:], bk_sb[:, t:t + 1])
            nc.vector.scalar_tensor_tensor(
                kscat(1), ps[:], bk_sb[:, t:t + 1], hi,
                op0=Alu.add, op1=Alu.subtract)

        def o_group(rc8, ob):
            ps = pspool.tile([P, NB], f32, tag="ps", name="ps")
            for cc in range(EC):
                nc.tensor.matmul(
                    ps[:],
                    att_sb[:, cc * QR + rc8 * P: cc * QR + (rc8 + 1) * P],
                    wot_sb[:, cc * EMB + ob * NB: cc * EMB + (ob + 1) * NB],
                    start=(cc == 0), stop=(cc == EC - 1))
            ev = evac.tile([P, NB], f32, tag="evo", name="ev")
            nc.vector.tensor_add(ev[:], ps[:], bob_sb[:, ob * NB:(ob + 1) * NB])
            nc.sync.dma_start(
                out[rc8 * P:(rc8 + 1) * P, ob * NB:(ob + 1) * NB], ev[:])

        # ---- warm-up: pair-0 critical path ------------------------------
        q_group(0, 0)
        k_group(0, 0)
        q_group(0, 1)
        k_group(0, 1)
        for rc in range(8):       # seq chunks 0..7 are the query half: no
            v_group(0, rc)        # dependence on the (later) key-column DMAs
        for ib in range(2):
            q_group(1, ib)

        # ---- filler queue (ordered; (needed_by_pair, emit_fn)) ----------
        fill = []

        def add_kq(t):
            fill.extend((t, lambda t=t, ib=ib: q_group(t, ib))
                        for ib in range(2))
            fill.extend((t, lambda t=t, jb=jb: k_group(t, jb))
                        for jb in range(4))

        def add_v(ob):
            fill.extend((max(2 * ob, 1), lambda ob=ob, rc=rc: v_group(ob, rc))
                        for rc in range(RC))

        fill.extend((1, lambda jb=jb: k_group(0, jb)) for jb in (2, 3))
        fill.extend((1, lambda rc=rc: v_group(0, rc)) for rc in range(8, RC))
        fill.extend((1, lambda jb=jb: k_group(1, jb)) for jb in range(4))
        add_kq(2)
        add_v(1)
        add_kq(3)
        add_v(2)
        add_kq(4)
        fill.append((5, load_wot))
        add_kq(5)
        add_v(3)
        add_kq(6)
        add_kq(7)
        fill_i = [0]

        def ensure_ready(t):
            while fill_i[0] < len(fill) and fill[fill_i[0]][0] <= t:
                fill[fill_i[0]][1]()
                fill_i[0] += 1

        def pull_filler():
            if fill_i[0] < len(fill):
                fill[fill_i[0]][1]()
                fill_i[0] += 1

        # ---- attention: one software-pipelined stream -------------------
        # PV runs one jc2 step behind exp so the PE never sits on the exp
        # latency; the stream crosses (e, ib) and pair boundaries without a
        # sync point (the next block's scores are emitted before the
        # previous block's last PV + normalize).
        steps = []
        for t in range(NP):
            # last pair runs ib-major so half the output projection can
            # start while its second i-block is still in softmax
            ebs = ([(e, ib) for e in range(2) for ib in range(QR // NB)]
                   if t < NP - 1 else
                   [(e, ib) for ib in range(QR // NB) for e in range(2)])
            steps += [(t, e, ib, jc2) for e, ib in ebs for jc2 in range(JC2)]

        kt3 = qv = None
        cur_t = -1
        blocks = {}   # (t, e, ib) -> ot_ps
        pend = None   # (t, e, ib, jc2, pt3)

        def finish_block(t, e, ib):
            # rows 0..63 = head output^T, row 64 = softmax denominator.
            # Broadcast the denominator row across 64 partitions via a DRAM
            # bounce, then a fused multiply evacuates + normalizes.
            h = 2 * t + e
            eb = e * HD
            ot_ps = blocks.pop((t, e, ib))
            rs = nrm.tile([P, NB], f32, tag="rs")
            nc.vector.reciprocal(rs[64:65, :], ot_ps[64:65, :])
            nc.gpsimd.dma_start(bscr[h, ib, :], rs[64:65, :])
            bc = nrm.tile([P, NB], f32, tag="bc")
            bsrc = bass.AP(bscr, (h * 2 + ib) * NB, [[0, HD], [1, NB]])
            nc.gpsimd.dma_start(bc[0:HD, :], bsrc)
            nc.vector.tensor_mul(
                att_sb[eb:eb + HD, t * QR + ib * NB: t * QR + (ib + 1) * NB],
                ot_ps[0:HD, :], bc[0:HD, :])
            if t == NP - 1 and e == 1:
                if ib == 0:
                    # first half of the output projection: feed through the
                    # filler queue so it interleaves with the ib=1 attention
                    # instead of blocking the score stream
                    fill.extend((99, lambda rc8=rc8, ob=ob: o_group(rc8, ob))
                                for rc8 in range(4) for ob in range(2))
                else:
                    for rc8 in range(4, 8):
                        for ob in range(2):
                            o_group(rc8, ob)

        def emit_pv(p):
            t, e, ib, jc2, pt = p
            for u in range(2):
                jc = jc2 * 2 + u
                ptu = bass.AP(pt.tensor, pt.offset + u * NB,
                              [list(pt.ap[0]), [0, 2], [1, NB]])
                nc.tensor.matmul(
                    blocks[(t, e, ib)][:, :], v_stat(2 * t + e, jc), ptu,
                    start=(jc == 0), stop=(jc == RC - 1),
                    perf_mode=DR)

        for t, e, ib, jc2 in steps:
            if t != cur_t:
                ensure_ready(t)
                ktp_t = ktp_tiles[t]
                qv = qall[:, t * 2 * QR:(t + 1) * 2 * QR] \
                    .rearrange("p (u n) -> p u n", u=2)
                cur_t = t
            if jc2 == 0:
                ot_new = otpool.tile([P, NB], f32, tag="ot", name="ot_new")
                blocks[(t, e, ib)] = ot_new

            eb = e * HD
            st_ps = stpool.tile([P, 2 * NB], f32, tag="st")
            for u in range(2):
                jc = jc2 * 2 + u
                ksl = ktp_t[eb:eb + HD, jc * 256:(jc + 1) * 256]
                nc.tensor.matmul(
                    st_ps[:, u * NB:(u + 1) * NB],
                    bass.AP(ksl.tensor, ksl.offset,
                            [list(ksl.ap[0]), [128, 2], [1, 128]]),
                    qv[eb:eb + HD, :, ib * NB:(ib + 1) * NB],
                    start=True, stop=True, perf_mode=DR)
            pt = ptpool.tile([P, 2 * NB], fp8, tag="pt")
            nc.scalar.activation(pt[:], st_ps[:], Exp, scale=SCALE)
            # filler BEFORE the pending PV: a pulled v_group must land ahead
            # of the first PV that consumes it
            if t == 0 and e == 0 and (ib == 0 or jc2 < 2):
                # front-fill: k0 jb2/3 + v rc8..15 must ALL be emitted
                # before PV(0,0,0,7) consumes the last v stripes
                pull_filler()
                if ib == 0 and jc2 == 0:
                    pull_filler()
            elif jc2 % 2 == 1:
                pull_filler()
            if pend is not None:
                emit_pv(pend)
                if pend[3] == JC2 - 1:
                    finish_block(*pend[:3])
            pend = (t, e, ib, jc2, pt)
        emit_pv(pend)
        while fill_i[0] < len(fill):    # drain any queued output-projection
            pull_filler()
        finish_block(*pend[:3])

    # gpsimd SWDGE queues need a ucode library selected; mirror
    # Bacc.insert_library_loads on this plain Bass module.
    import bass_rust as _bass_rust
    from concourse.library_config import all_libraries, standard

    inst_type_to_lib_mask = {}
    for lib in all_libraries:
        for inst_type in lib.instructions:
            inst_type_to_lib_mask[inst_type] = inst_type_to_lib_mask.get(
                inst_type, 0) | (1 << lib.index)
    _bass_rust.insert_library_loads(
        nc, inst_type_to_lib_mask, len(all_libraries), standard.index)

    _split_multi_waits(nc, mybir)

    return nc


def _split_multi_waits(nc, mybir):
    """This walrus build accepts at most ONE sync-wait per instruction; Tile
    emits several.  Hoist all but the last wait onto single-wait NoOps placed
    immediately before the instruction on the same engine."""
    nop_id = [0]
    for fn in nc.m.functions:
        for bb in fn.blocks:
            out = []
            for inst in bb.instructions:
                si = inst.sync_info
                if si is not None and si.on_wait is not None \
                        and len(si.on_wait) > 1:
                    waits = list(si.on_wait)
                    for w in waits[:-1]:
                        nop = mybir.InstNoOp(
                            name=f"I-waitsplit-{nop_id[0]}", ins=[], outs=[])
                        nop_id[0] += 1
                        nop.engine = inst.engine
                        nop.sync_info = mybir.SyncInfo(
                            on_wait=[w], on_update=[])
                        out.append(nop)
                    inst.sync_info = mybir.SyncInfo(
                        on_wait=[waits[-1]],
                        on_update=list(si.on_update or []))
                out.append(inst)
            bb.instructions = out


def _get_compiled():
    global _COMPILED
    if _COMPILED is None:
        _COMPILED = _build()
    return _COMPILED


def kernel(x, wq, bq, wk, bk, wv, bv, wo, bo, _want_results_obj=False,
           **run_kwargs):
    import ml_dtypes
    from concourse.bass_utils import run_bass_kernel_spmd

    bf = ml_dtypes.bfloat16
    x = np.asarray(x, dtype=np.float32)
    wq = np.asarray(wq, dtype=np.float32)
    bq = np.asarray(bq, dtype=np.float32)
    wk = np.asarray(wk, dtype=np.float32)
    bk = np.asarray(bk, dtype=np.float32)
    wv = np.asarray(wv, dtype=np.float32)
    bv = np.asarray(bv, dtype=np.float32)
    wo = np.asarray(wo, dtype=np.float32)
    bo = np.asarray(bo, dtype=np.float32)

    bs, seq, emb = x.shape
    assert (bs, seq, emb) == (4, SEQ, EMB)

    nc = _get_compiled()

    shared = {
        "wqt": np.ascontiguousarray(wq.T).astype(bf),
        "wkt": np.ascontiguousarray(wk.T).astype(bf),
        "wvt": np.ascontiguousarray(wv.T).astype(bf),
        "wot": np.ascontiguousarray(wo.T).astype(bf),
        "bqp": np.ascontiguousarray(bq.reshape(EC, P).T),
        "bkp": np.ascontiguousarray(bk.reshape(EC, P).T),
        "bob": np.ascontiguousarray(
            np.broadcast_to(bo + wo @ bv, (P, EMB))),
    }
    in_maps = []
    for c in range(8):
        b, hf = c // 2, c % 2
        xb = x[b]
        # this core's query rows first; row order of keys/values is irrelevant
        xb_perm = np.concatenate(
            [xb[hf * QR:(hf + 1) * QR], xb[(1 - hf) * QR:(2 - hf) * QR]], axis=0)
        in_maps.append({
            "xt": np.ascontiguousarray(xb_perm.T).astype(bf),
            **shared,
        })

    res = run_bass_kernel_spmd(nc, in_maps, core_ids=list(range(8)),
                               **run_kwargs)

    outp = np.empty((bs, seq, emb), dtype=np.float32)
    for c in range(8):
        b, hf = c // 2, c % 2
        outp[b, hf * QR:(hf + 1) * QR, :] = res.results[c]["out"]
    if _want_results_obj:
        return outp, res
    return outp
